# revision 13
# baseline (speedup 1.0000x reference)
"""CLIP contrastive loss on 8 Trainium2 NeuronCores.

Math (reference): with n = 4096, 2n = 8192 rows of L2-normalized features,
  logits_per_image = scale * img[:n] @ txt.T        [n, 2n]
  logits_per_text  = scale * txt[:n] @ img.T        [n, 2n]
  loss = (ce(logits_per_image) + ce(logits_per_text)) / 2,
  ce(L) = mean_r(logsumexp(L[r]) - L[r, r]).

Both CE terms are slices of the single matrix M = scale * img @ txt.T:
logits_per_image = M[:n, :] and logits_per_text = M[:, :n].T, so the
shared block M[:n, :n] is computed ONCE (3n^2 dot products instead of 4n^2):

  region T = M[:n, :]     -> row sums of exp give the image-CE denominators;
                             exp of the first n columns is also kept (bf16)
                             for the text-CE column sums.
  region B = M[n:, :n]    -> exp kept (bf16) for the text-CE column sums.

Distribution: core c owns rows [c*512, (c+1)*512) of BOTH regions (img rows
c*512.. for T, n + c*512.. for B).  Matmuls run in fp8(e4m3) DoubleRow perf
mode (157 TF/s); ACT fuses exp with the T row sums (accum_out).  The eight
[128, cw] bf16 exp tiles per column chunk (4 T m-blocks + 4 B m-blocks) are
tree-added on the (otherwise idle) DVE at its 2x bf16 rate into a single
comb[128, 4096] tile that is DMAed out; the host finishes the 128-partition
column reduction, the diagonal (exact, f64), and the final mean.

Features are quantized host-side: stationary = e4m3(img * scale/16),
moving = e4m3(txt * 16), so psum = scale * <img, txt> directly.  The moving
tensor is staged in DRAM as one tensor PER CHUNK shaped like its SBUF tile,
so every partition's chunk data is a single contiguous KC*cw-byte run and
the DMA moves ~8 KB packets (small packets cap a ring at ~70 GB/s and pace
the whole kernel otherwise).
"""

import numpy as np
import ml_dtypes

import concourse.tile as tile
from concourse import bacc, mybir
from concourse.bass_utils import run_bass_kernel_spmd

TWO_N = 8192   # total rows (and T logits columns)
N = 4096       # CE rows
D = 512        # embedding dim
C = 8          # cores
R = N // C     # CE rows per core = 512
KC = D // 128  # contraction k-tiles = 4
W = 2048       # widest column chunk (psum tile width)
MB = R // 128  # 128-row blocks per core = 4
QSCALE = 16.0  # feature pre-scale before e4m3 quantization

BF16 = mybir.dt.bfloat16
FP8 = mybir.dt.float8e4
F32 = mybir.dt.float32

_CACHE = {}

# T-region column chunks: the first n=4096 columns (shared with B) as
# [256, 1792, 2048] (narrow lead-in starts ACT early), then the row-only
# half as [2048, 2048].
SHARED_CHUNKS = [256, 1792, 2048]
TAIL_CHUNKS = [2048, 2048]
T_CHUNKS = SHARED_CHUNKS + TAIL_CHUNKS
PCOLS = len(T_CHUNKS)  # partials free columns (T row-sum pieces per m)
# Chunk job order: B jobs interleaved between T jobs so the PE always has
# matmul work while ACT drains the previous chunk's psum (a strict T-then-B
# order stalls the PE ~7us at the transition waiting for psum buffers), and
# the kernel ends on the narrow 256-wide B job for a short pipeline drain.
JOB_ORDER = [("T", 0), ("T", 1), ("T", 2), ("B", 2), ("T", 3), ("B", 1), ("T", 4), ("B", 0)]


def _build():
    """Build the (core-uniform) Bass/Tile program once."""
    nc = bacc.Bacc("TRN2", target_bir_lowering=False, debug=False, num_devices=C)

    stat1 = nc.dram_tensor("stat1", [128, KC, R], FP8, kind="ExternalInput").ap()
    stat2 = nc.dram_tensor("stat2", [128, KC, R], FP8, kind="ExternalInput").ap()
    movs = [
        nc.dram_tensor(f"mov{ci}", [128, KC, cw], FP8, kind="ExternalInput").ap()
        for ci, cw in enumerate(T_CHUNKS)
    ]
    out = nc.dram_tensor("out", [128, MB, PCOLS], F32, kind="ExternalOutput").ap()
    outc = nc.dram_tensor("outc", [128, N], BF16, kind="ExternalOutput").ap()

    DR = mybir.MatmulPerfMode.DoubleRow
    EXP = mybir.ActivationFunctionType.Exp
    add = mybir.AluOpType.add

    t_off = [0]
    for cw in T_CHUNKS:
        t_off.append(t_off[-1] + cw)

    with tile.TileContext(nc) as tc:
        with (
            tc.tile_pool(name="stat", bufs=1) as stat_pool,
            tc.tile_pool(name="acc", bufs=1) as acc_pool,
            tc.tile_pool(name="mov", bufs=1) as mov_pool,
            tc.tile_pool(name="exp", bufs=1) as exp_pool,
            tc.tile_pool(name="red", bufs=1) as red_pool,
            tc.tile_pool(name="psum", bufs=2, space="PSUM") as psum_pool,
        ):
            st1 = stat_pool.tile([128, KC, R], FP8, tag="st1")
            st2 = stat_pool.tile([128, KC, R], FP8, tag="st2")

            # PE warm-up: a few throwaway matmuls with no DMA deps start the
            # HAM clock ramp (0.65 -> 2.4 GHz) while the first chunk streams.
            warm = stat_pool.tile([128, 512], BF16, tag="warm")
            nc.vector.memset(warm[:], 0.0)
            wps = psum_pool.tile([128, W], F32, tag="ps")
            for _ in range(4):
                nc.tensor.matmul(
                    wps[:, 0:512], warm[:, 0:128], warm[:, 0:512],
                    start=True, stop=True,
                )

            # partials[p, m, i] = sum_j exp(T[m-block row p, chunk i cols j])
            partials = acc_pool.tile([128, MB, PCOLS], F32, tag="partials")
            nc.vector.memset(partials[:], 0.0)

            # stationaries ride the gpsimd HWDGE ring, streaming in parallel
            # with chunk0 on the sync ring
            nc.gpsimd.dma_start(st1[:], stat1[:])
            nc.gpsimd.dma_start(st2[:], stat2[:])

            mov_tiles = {}   # ci -> (tile, cw) for shared chunks (kept for B)
            exp_t = {}       # (region, ci, m) -> bf16 exp tile
            u_t = {}         # ci -> early T-side tree sum (xT0+xT1+xT2+xT3)

            def mm_block(ps, st, mt, m, cw):
                for so in range(0, cw, 512):
                    sw = min(512, cw - so)
                    for k in (0, 2):
                        nc.tensor.matmul(
                            ps[:, so:so + sw],
                            st[:, k:k + 2, m * 128:(m + 1) * 128],
                            mt[:, k:k + 2, so:so + sw],
                            start=(k == 0),
                            stop=(k == 2),
                            perf_mode=DR,
                        )

            for region, ci in JOB_ORDER:
                if region == "T":
                    cw = T_CHUNKS[ci]
                    shared = ci < len(SHARED_CHUNKS)
                    if shared:
                        mt = mov_pool.tile([128, KC, cw], FP8, tag=f"ms{ci}")
                        mov_tiles[ci] = (mt, cw)
                    else:
                        mt = mov_pool.tile([128, KC, W], FP8, tag=f"mt{ci}")
                    # alternate chunks across the two HWDGE rings so chunk
                    # i+1 streams while chunk i transfers (one ring tops out
                    # ~110-190 GB/s and paces ACT otherwise)
                    ring = nc.sync if ci % 2 == 0 else nc.gpsimd
                    ring.dma_start(mt[:, :, 0:cw], movs[ci][:])
                    for m in range(MB):
                        ps = psum_pool.tile([128, W], F32, tag="ps")
                        mm_block(ps, st1, mt, m, cw)
                        if shared:
                            xt = exp_pool.tile([128, cw], BF16, tag=f"xT{ci}_{m}")
                            exp_t[("T", ci, m)] = xt
                            out_ap = xt[:, 0:cw]
                        else:
                            out_ap = ps[:, 0:cw]
                        nc.scalar.activation(
                            out_ap, ps[:, 0:cw], EXP, bias=0.0,
                            accum_out=partials[:, m, ci:ci + 1],
                        )
                    if shared:
                        # early T-side tree: u = (m0+m1)+(m2+m3) on DVE so the
                        # end-of-kernel chain after the last B exp is short
                        t0, t1, t2, t3 = (exp_t[("T", ci, m)] for m in range(MB))
                        a01 = red_pool.tile([128, cw], BF16, tag=f"a01_{ci}")
                        a23 = red_pool.tile([128, cw], BF16, tag=f"a23_{ci}")
                        u = red_pool.tile([128, cw], BF16, tag=f"u_{ci}")
                        u_t[ci] = u
                        nc.vector.tensor_tensor(out=a01[:], in0=t0[:], in1=t1[:], op=add)
                        nc.vector.tensor_tensor(out=a23[:], in0=t2[:], in1=t3[:], op=add)
                        nc.vector.tensor_tensor(out=u[:], in0=a01[:], in1=a23[:], op=add)
                else:
                    mt, cw = mov_tiles[ci]
                    for m in range(MB):
                        ps = psum_pool.tile([128, W], F32, tag="ps")
                        mm_block(ps, st2, mt, m, cw)
                        xb = exp_pool.tile([128, cw], BF16, tag=f"xB{ci}_{m}")
                        exp_t[("B", ci, m)] = xb
                        nc.scalar.activation(xb[:, 0:cw], ps[:, 0:cw], EXP, bias=0.0)
                    b0, b1, b2, b3 = (exp_t[("B", ci, m)] for m in range(MB))
                    b01 = red_pool.tile([128, cw], BF16, tag=f"b01_{ci}")
                    b23 = red_pool.tile([128, cw], BF16, tag=f"b23_{ci}")
                    comb = red_pool.tile([128, cw], BF16, tag=f"cb_{ci}")
                    nc.vector.tensor_tensor(out=b01[:], in0=b0[:], in1=b1[:], op=add)
                    nc.vector.tensor_tensor(out=b23[:], in0=b2[:], in1=b3[:], op=add)
                    nc.vector.tensor_tensor(out=b01[:], in0=b01[:], in1=b23[:], op=add)
                    nc.vector.tensor_tensor(out=comb[:], in0=u_t[ci][:], in1=b01[:], op=add)
                    nc.gpsimd.dma_start(outc[:, t_off[ci]:t_off[ci] + cw], comb[:])

            # T row-sum partials out on the sync ring (tiny, 16KB)
            nc.sync.dma_start(out[:], partials[:])

    nc.compile()
    return nc


def _get_nc():
    if "nc" not in _CACHE:
        _CACHE["nc"] = _build()
    return _CACHE["nc"]


def _prep_inputs(image_features, text_features, logit_scale):
    img = np.asarray(image_features, dtype=np.float32)
    txt = np.asarray(text_features, dtype=np.float32)
    scale = float(np.asarray(logit_scale, dtype=np.float32))

    # mov{ci}[p, k, j] = QSCALE * txt[t_off[ci] + j, k*128 + p], e4m3
    a = np.ascontiguousarray(txt.T * np.float32(QSCALE)).reshape(KC, 128, TWO_N)
    movf = a.transpose(1, 0, 2).astype(ml_dtypes.float8_e4m3)  # [128, KC, 2N]
    mov_chunks = {}
    off = 0
    for ci, cw in enumerate(T_CHUNKS):
        mov_chunks[f"mov{ci}"] = np.ascontiguousarray(movf[:, :, off:off + cw])
        off += cw

    def stat_layout(rows):
        # [p, k, m] = (scale/QSCALE) * feat[row0 + m, k*128 + p], e4m3
        a = (rows * np.float32(scale / QSCALE)).T.reshape(KC, 128, R)
        return np.ascontiguousarray(a.transpose(1, 0, 2).astype(ml_dtypes.float8_e4m3))

    in_maps = [
        {
            "stat1": stat_layout(img[c * R:(c + 1) * R]),
            "stat2": stat_layout(img[N + c * R:N + (c + 1) * R]),
            **mov_chunks,
        }
        for c in range(C)
    ]
    # diagonal logits (same for both CE terms): scale * <img_r, txt_r>
    diag = scale * np.sum(
        img[:N].astype(np.float64) * txt[:N].astype(np.float64), axis=1
    )
    return in_maps, diag


def _finish(results, diag):
    # image CE: S_img[c*R + m*128 + p] = sum_i out[c][p, m, i]
    s = np.stack([results[c]["out"] for c in range(C)]).astype(np.float64)
    s_img = s.sum(axis=-1)  # [c, p, m]
    rows = (
        np.arange(C)[:, None, None] * R
        + np.arange(MB)[None, None, :] * 128
        + np.arange(128)[None, :, None]
    )  # [c, p, m]
    ce_img = np.mean(np.log(s_img) - diag[rows])
    # text CE: S_txt[j] = sum_c sum_p comb[c][p, j]
    combs = np.stack([np.asarray(results[c]["outc"], dtype=np.float64) for c in range(C)])
    s_txt = combs.sum(axis=(0, 1))  # [N]
    ce_txt = np.mean(np.log(s_txt) - diag)
    return np.float32((ce_img + ce_txt) / 2.0)


def kernel(image_features, text_features, logit_scale):
    nc = _get_nc()
    in_maps, diag = _prep_inputs(image_features, text_features, logit_scale)
    res = run_bass_kernel_spmd(nc, in_maps, list(range(C)))
    return _finish(res.results, diag)


if __name__ == "__main__":
    rng = np.random.default_rng(0)
    img = rng.standard_normal((TWO_N, D), dtype=np.float32)
    txt = rng.standard_normal((TWO_N, D), dtype=np.float32)
    img /= np.linalg.norm(img, axis=-1, keepdims=True)
    txt /= np.linalg.norm(txt, axis=-1, keepdims=True)
    print(kernel(img, txt, np.float32(100.0)))
